# revision 3
# baseline (speedup 1.0000x reference)
"""Trainium2 Bass kernel for nn_Mlp_13099650253522 (BitNet-ternary dense MLP).

  h = gelu(x @ ter_quant(w1).T + b1);  y = h @ ter_quant(w2).T + b2
  ter_quant(w) = clip(round(w / g), -1, 1) * g,  g = mean(|w|) + 1e-5

Strategy (8 NeuronCores, data-parallel over the 64*197=12608 tokens,
1576 tokens/core). Schedule is built to keep the PE busy end-to-end:

 - PE warmup: ~90 dummy matmuls from t=0 hold the HAM clock gate at
   2.4 GHz so real matmuls never run cold.
 - gamma passes read a bf16 copy of each weight matrix (numerically
   safe: flip fraction ~1e-6); the fp32 weights then stream exactly
   once with ternary quantization fused on arrival (no SBUF residency,
   no double-read).
 - fc1 runs as two half-passes (tokens {0,1} then {2,3} per weight
   block) so block arrival (~1.1us) stays ahead of PE consumption
   (~2us/block). w2's gamma pass + fp32 stream + quant all overlap
   fc1; fc2 then runs back-to-back with everything resident.
 - Quant ops alternate DVE/ACT and are emitted interleaved with the
   fc1 loops to keep both strict-FIFO engine queues inversion-free.
 - Weights+x stream on the sync DMA queue (explicitly chained); y
   tiles leave on the gpsimd queue.
"""
import sys

for _p in ("/root/.axon_site", "/root/.axon_site/_ro/trn_rl_repo",
           "/root/.axon_site/_ro/pypackages", "/opt/trn_rl_repo"):
    if _p not in sys.path:
        sys.path.append(_p)

import ml_dtypes
import numpy as np

from concourse import bacc
import concourse.mybir as mybir
from concourse import bass_isa
from concourse.tile import TileContext
from concourse.tile_rust import add_dep_helper
from concourse.bass_utils import run_bass_kernel_spmd

FP32 = mybir.dt.float32
BF16 = mybir.dt.bfloat16
FP8 = mybir.dt.float8e4
Act = mybir.ActivationFunctionType
Alu = mybir.AluOpType
AxX = mybir.AxisListType.X
AxXY = mybir.AxisListType.XY

N_CORES = 8
B, S, D, H = 64, 197, 768, 3072
TOK = B * S                 # 12608
TOK_PER = TOK // N_CORES    # 1576
NT = 4                      # token tiles per core
TN = TOK_PER // NT          # 394
KD = D // 128               # 6
KH = H // 128               # 24
EPS = 1e-5
NDUM = 88                   # PE warmup matmuls (~15us span)


def build():
    nc = bacc.Bacc("TRN2", target_bir_lowering=False, debug=False)
    # w1 in hc-blocked layout: row hc*128+k, col kd*128+m  ==  w1[hc*128+m, kd*128+k]
    w1g = nc.declare_dram_parameter("w1g", [H, D], BF16, isOutput=False)
    w1p = nc.declare_dram_parameter("w1p", [H, D], FP32, isOutput=False)
    # x in (t,kd)-blocked layout: row (t*6+kd)*128+k, col n == x_core[t*394+n, kd*128+k]
    xt = nc.declare_dram_parameter("xt", [H, TN], BF16, isOutput=False)
    # w2 = w2.T (natural): row kh*128+k, col dc*128+m == w2[dc*128+m, kh*128+k]
    w2g = nc.declare_dram_parameter("w2g", [H, D], BF16, isOutput=False)
    w2p = nc.declare_dram_parameter("w2p", [H, D], FP32, isOutput=False)
    b1r = nc.declare_dram_parameter("b1r", [128, KH], FP32, isOutput=False)
    b2r = nc.declare_dram_parameter("b2r", [128, KD], FP32, isOutput=False)
    # y out: row (t*6+dc)*128+k, col n == y_core[t*394+n, dc*128+k]
    yt = nc.declare_dram_parameter("yt", [H, TN], FP32, isOutput=True)

    with TileContext(nc) as tc:
        with (
            tc.tile_pool(name="singles", bufs=1) as singles,
            tc.tile_pool(name="gslab", bufs=2) as gslabp,     # [128,6,768] bf16
            tc.tile_pool(name="wring", bufs=5) as wringp,     # [128,768] fp32 stream
            tc.tile_pool(name="sgnA", bufs=4) as sgnAp,
            tc.tile_pool(name="sgnD", bufs=4) as sgnDp,
            tc.tile_pool(name="t1", bufs=KH) as t1p,          # w1 ternary fp8, resident
            tc.tile_pool(name="t2", bufs=KH) as t2p,          # w2 ternary fp8, resident
            tc.tile_pool(name="xb", bufs=NT) as xbp,          # x bf16, resident
            tc.tile_pool(name="hb", bufs=KH * NT) as hbp,     # gelu out bf16, resident
            tc.tile_pool(name="ysb", bufs=4) as ysbp,
            tc.tile_pool(name="psw", bufs=1, space="PSUM") as pswp,
            tc.tile_pool(name="ps1", bufs=4, space="PSUM") as ps1p,
            tc.tile_pool(name="ps2", bufs=3, space="PSUM") as ps2p,
        ):
            # ---- gpsimd library pre-warm (partition_all_reduce is a custom op)
            dmy = singles.tile([128, 1], FP32, tag="dmy")
            nc.gpsimd.memset(dmy, 0.0)
            dmy2 = singles.tile([128, 1], FP32, tag="dmy2")
            nc.gpsimd.partition_all_reduce(dmy2, dmy, channels=128,
                                           reduce_op=bass_isa.ReduceOp.add)

            # ---- biases (gpsimd queue; tiny)
            b1sb = singles.tile([128, KH], FP32, tag="b1sb")
            nc.gpsimd.dma_start(out=b1sb, in_=b1r[:, :])
            b2sb = singles.tile([128, KD], FP32, tag="b2sb")
            nc.gpsimd.dma_start(out=b2sb, in_=b2r[:, :])

            # ---- PE warmup: keep HAM at K=8/8 until real matmuls arrive
            dmw = singles.tile([128, TN], BF16, tag="dmw")
            nc.vector.memset(dmw, 0.0)
            wps = pswp.tile([128, TN], FP32, tag="wps")
            for _ in range(NDUM):
                nc.tensor.matmul(wps, dmw[:, 0:128], dmw, start=True, stop=True)

            chain = [None]

            def dma_chained(q, out, in_):
                dma = q.dma_start(out=out, in_=in_)
                if chain[0] is not None:
                    add_dep_helper(dma.ins, chain[0].ins, reason="dma order")
                chain[0] = dma
                return dma

            def gamma_chain(acc, n_cols, tag):
                """|w| col-sums [128,n] -> (g/2, -g/2) broadcast tiles [128,1]"""
                rsum = singles.tile([128, 1], FP32, tag=tag + "_rs")
                nc.vector.tensor_reduce(out=rsum[:, 0:1], in_=acc[:, 0:n_cols],
                                        axis=AxX, op=Alu.add)
                allr = singles.tile([128, 1], FP32, tag=tag + "_ar")
                nc.gpsimd.partition_all_reduce(allr, rsum, channels=128,
                                               reduce_op=bass_isa.ReduceOp.add)
                gf = singles.tile([128, 1], FP32, tag=tag + "_gf")
                nc.vector.tensor_scalar(
                    out=gf, in0=allr, scalar1=1.0 / (D * H),
                    scalar2=EPS, op0=Alu.mult, op1=Alu.add)
                gh = singles.tile([128, 1], FP32, tag=tag + "_gh")
                nc.vector.tensor_scalar_mul(gh, gf, 0.5)
                gn = singles.tile([128, 1], FP32, tag=tag + "_gn")
                nc.vector.tensor_scalar_mul(gn, gf, -0.5)
                return gh, gn

            def gamma_pass(gdram, acc_tag):
                """bf16 pre-read of a [H,D] matrix -> per-partition |w| sums"""
                acc = singles.tile([128, 4], FP32, tag=acc_tag)
                for i in range(4):
                    gt = gslabp.tile([128, KD, D], BF16, tag="gslab")
                    dma_chained(nc.sync, gt,
                                gdram[i * 768:(i + 1) * 768, :]
                                .rearrange("(b p) f -> p b f", p=128))
                    nc.vector.tensor_reduce(out=acc[:, i:i + 1], in_=gt,
                                            axis=AxXY, op=Alu.add,
                                            apply_absolute_value=True)
                return acc

            def quant_act(wf, t, gh, gn):
                a = sgnAp.tile(list(wf.shape), FP8, tag="sgnA")
                b = sgnAp.tile(list(wf.shape), FP8, tag="sgnA")
                nc.scalar.activation(a, wf, Act.Sign, bias=gh[:, 0:1])
                nc.scalar.activation(b, wf, Act.Sign, bias=gn[:, 0:1])
                nc.vector.tensor_add(out=t, in0=a, in1=b)

            def quant_dve(wf, t, gh, gn):
                a = sgnDp.tile(list(wf.shape), FP8, tag="sgnD")
                b = sgnDp.tile(list(wf.shape), FP8, tag="sgnD")
                nc.vector.tensor_scalar(out=a, in0=wf, scalar1=gh[:, 0:1],
                                        scalar2=2.0, op0=Alu.is_ge,
                                        op1=Alu.mult)
                nc.vector.tensor_scalar(out=b, in0=wf, scalar1=gn[:, 0:1],
                                        scalar2=2.0, op0=Alu.is_le,
                                        op1=Alu.mult)
                nc.vector.tensor_sub(out=t, in0=a, in1=b)

            # ---- gamma1 from bf16 pre-pass
            acc1 = gamma_pass(w1g, "acc1")
            g1h, g1n = gamma_chain(acc1, 4, "g1")

            # ---- w1 fp32 stream + x loads (chained on sync queue)
            w1f = [None] * KH
            wf0 = wringp.tile([128, D], FP32, tag="wf")
            w1f[0] = wf0
            dma_chained(nc.sync, w1f[0], w1p[0:128, :])

            xb = [None] * NT
            for t in (0, 1):
                xbt = xbp.tile([128, KD, TN], BF16, tag="xb")
                xb[t] = xbt
                dma_chained(nc.sync, xb[t],
                            xt[t * 768:(t + 1) * 768, :]
                            .rearrange("(b p) f -> p b f", p=128))

            for hc in range(1, KH):
                wfh = wringp.tile([128, D], FP32, tag="wf")
                w1f[hc] = wfh
                dma_chained(nc.sync, w1f[hc],
                            w1p[hc * 128:(hc + 1) * 128, :])

            # ---- quant + fc1 interleaved (lookahead keeps FIFOs clean)
            t1 = [None] * KH

            def quant1(hc):
                t = t1p.tile([128, D], FP8, tag="t1")
                if hc % 2 == 0:
                    quant_dve(w1f[hc], t, g1h, g1n)
                else:
                    quant_act(w1f[hc], t, g1h, g1n)
                t1[hc] = t

            hbt = {}

            def fc1(hc, ts):
                for t in ts:
                    ps = ps1p.tile([128, TN], FP32, tag="ps1")
                    for kd in range(KD):
                        nc.tensor.matmul(ps, t1[hc][:, kd * 128:(kd + 1) * 128],
                                         xb[t][:, kd, :],
                                         start=(kd == 0), stop=(kd == KD - 1))
                    hbv = hbp.tile([128, TN], BF16, tag="hb")
                    nc.scalar.activation(hbv, ps, Act.Gelu,
                                         bias=b1sb[:, hc:hc + 1],
                                         scale=g1h[:, 0:1])
                    hbt[(hc, t)] = hbv

            LOOK1 = 3
            for hc in range(LOOK1):
                quant1(hc)
            for hc in range(KH):
                if hc + LOOK1 < KH:
                    quant1(hc + LOOK1)
                fc1(hc, (0, 1))

            # ---- late x tiles, gamma2 pre-pass, w2 fp32 stream (all overlap fc1)
            for t in (2, 3):
                xbt = xbp.tile([128, KD, TN], BF16, tag="xb")
                xb[t] = xbt
                dma_chained(nc.sync, xb[t],
                            xt[t * 768:(t + 1) * 768, :]
                            .rearrange("(b p) f -> p b f", p=128))

            acc2 = gamma_pass(w2g, "acc2")
            g2h, g2n = gamma_chain(acc2, 4, "g2")

            w2f = [None] * KH
            for kh in range(KH):
                wfk = wringp.tile([128, D], FP32, tag="wf")
                w2f[kh] = wfk
                dma_chained(nc.sync, w2f[kh],
                            w2p[kh * 128:(kh + 1) * 128, :])

            t2 = [None] * KH

            def quant2(kh):
                t = t2p.tile([128, D], FP8, tag="t2")
                if kh % 2 == 0:
                    quant_dve(w2f[kh], t, g2h, g2n)
                else:
                    quant_act(w2f[kh], t, g2h, g2n)
                t2[kh] = t

            LOOK2 = 2
            for kh in range(LOOK2):
                quant2(kh)
            for hc in range(KH):
                if hc + LOOK2 < KH:
                    quant2(hc + LOOK2)
                fc1(hc, (2, 3))

            # ---- fc2: everything resident, PE back-to-back; y out on gpsimd q
            for t in range(NT):
                for dc in range(KD):
                    ps = ps2p.tile([128, TN], FP32, tag="ps2")
                    for kh in range(KH):
                        nc.tensor.matmul(ps, t2[kh][:, dc * 128:(dc + 1) * 128],
                                         hbt[(kh, t)],
                                         start=(kh == 0), stop=(kh == KH - 1))
                    ysb = ysbp.tile([128, TN], FP32, tag="ysb")
                    nc.vector.tensor_scalar(
                        out=ysb, in0=ps, scalar1=g2h[:, 0:1],
                        scalar2=b2sb[:, dc:dc + 1],
                        op0=Alu.mult, op1=Alu.add)
                    nc.gpsimd.dma_start(
                        out=yt[(t * KD + dc) * 128:(t * KD + dc + 1) * 128, :],
                        in_=ysb)

    nc.compile()
    return nc


_NC = None


def _get_nc():
    global _NC
    if _NC is None:
        _NC = build()
    return _NC


def kernel(x, w1, b1, w2, b2, _trace=False, _trace_kwargs=None):
    nc = _get_nc()
    x = np.asarray(x, dtype=np.float32)
    w1 = np.asarray(w1, dtype=np.float32)
    b1 = np.asarray(b1, dtype=np.float32)
    w2 = np.asarray(w2, dtype=np.float32)
    b2 = np.asarray(b2, dtype=np.float32)

    # w1 -> hc-blocked layout [3072, 768]: row hc*128+k, col kd*128+m
    w1p = np.ascontiguousarray(
        w1.reshape(KH, 128, KD, 128).transpose(0, 3, 2, 1).reshape(H, D))
    w1g = w1p.astype(ml_dtypes.bfloat16)
    w2p = np.ascontiguousarray(w2.T)                    # [3072, 768]
    w2g = w2p.astype(ml_dtypes.bfloat16)
    b1r = np.ascontiguousarray(b1.reshape(KH, 128).T)   # [128, 24]
    b2r = np.ascontiguousarray(b2.reshape(KD, 128).T)   # [128, 6]

    x2 = x.reshape(TOK, D)
    in_maps = []
    for c in range(N_CORES):
        xc = x2[c * TOK_PER:(c + 1) * TOK_PER]          # [1576, 768]
        xtc = np.ascontiguousarray(
            xc.reshape(NT, TN, KD, 128).transpose(0, 2, 3, 1).reshape(H, TN)
        ).astype(ml_dtypes.bfloat16)
        in_maps.append({
            "xt": xtc, "w1g": w1g, "w1p": w1p, "w2g": w2g, "w2p": w2p,
            "b1r": b1r, "b2r": b2r,
        })
    out = run_bass_kernel_spmd(nc, in_maps, list(range(N_CORES)),
                               trace=_trace, **(_trace_kwargs or {}))
    res = out.results
    y = np.empty((TOK, D), dtype=np.float32)
    for c in range(N_CORES):
        ytc = res[c]["yt"]                              # [3072, 394]
        y[c * TOK_PER:(c + 1) * TOK_PER] = (
            ytc.reshape(NT, KD, 128, TN).transpose(0, 3, 1, 2).reshape(TOK_PER, D))
    y = y.reshape(B, S, D)
    if _trace:
        return y, out
    return y


# revision 4
# speedup vs baseline: 1.3943x; 1.3943x over previous
"""Trainium2 Bass kernel for nn_Mlp_13099650253522 (BitNet-ternary dense MLP).

  h = gelu(x @ ter_quant(w1).T + b1);  y = h @ ter_quant(w2).T + b2
  ter_quant(w) = clip(round(w / g), -1, 1) * g,  g = mean(|w|) + 1e-5

Strategy (8 NeuronCores, data-parallel over the 64*197=12608 tokens,
1576 tokens/core). Schedule is built to keep the PE busy end-to-end:

 - PE warmup: dummy matmuls (one long accumulation group, no WAW
   stalls) from t=0 hold the HAM clock gate at 2.4 GHz so real matmuls
   never run cold.
 - gamma passes read a bf16 copy of each weight matrix (numerically
   safe: flip fraction ~1e-6) in host-preswizzled partition-major
   slabs; the fp32 weights then stream exactly once, two 128-row
   blocks per DMA, with ternary quantization fused on arrival (no
   SBUF residency, no double-read).
 - fc1 runs as two half-passes (tokens {0,1} then {2,3} per weight
   block) so block arrival stays ahead of PE consumption. w2's gamma
   pass + fp32 stream + quant all overlap fc1; fc2 then runs
   back-to-back with everything resident.
 - All quant on DVE, all gelu on ACT: strict-FIFO queues stay
   inversion-free. DMAs are NOT cross-chained (queue ring order
   already serializes them; explicit deps would kill pipelining).
 - Weights+x stream on the sync DMA queue; y tiles leave on gpsimd.
"""
import sys

for _p in ("/root/.axon_site", "/root/.axon_site/_ro/trn_rl_repo",
           "/root/.axon_site/_ro/pypackages", "/opt/trn_rl_repo"):
    if _p not in sys.path:
        sys.path.append(_p)

import ml_dtypes
import numpy as np

from concourse import bacc
import concourse.mybir as mybir
from concourse import bass_isa
from concourse.tile import TileContext
from concourse.bass_utils import run_bass_kernel_spmd

FP32 = mybir.dt.float32
BF16 = mybir.dt.bfloat16
FP8 = mybir.dt.float8e4
Act = mybir.ActivationFunctionType
Alu = mybir.AluOpType
AxX = mybir.AxisListType.X
AxXY = mybir.AxisListType.XY

N_CORES = 8
B, S, D, H = 64, 197, 768, 3072
TOK = B * S                 # 12608
TOK_PER = TOK // N_CORES    # 1576
NT = 4                      # token tiles per core
TN = TOK_PER // NT          # 394
KD = D // 128               # 6
KH = H // 128               # 24
NP = KH // 2                # 12 weight block-pairs per matrix
EPS = 1e-5
NDUM = 92                   # PE warmup matmuls


def build():
    nc = bacc.Bacc("TRN2", target_bir_lowering=False, debug=False)
    # bf16 gamma copies, partition-major slabs: [128, slab, 6*768]
    w1g = nc.declare_dram_parameter("w1g", [128, 4, KD * D], BF16, isOutput=False)
    w2g = nc.declare_dram_parameter("w2g", [128, 4, KD * D], BF16, isOutput=False)
    # w1 in hc-blocked layout: row hc*128+k, col kd*128+m == w1[hc*128+m, kd*128+k]
    w1p = nc.declare_dram_parameter("w1p", [H, D], FP32, isOutput=False)
    # w2 = w2.T (natural): row kh*128+k, col dc*128+m == w2[dc*128+m, kh*128+k]
    w2p = nc.declare_dram_parameter("w2p", [H, D], FP32, isOutput=False)
    # x partition-major: [128, t, kd*394]; [k, t, kd*TN+n] == x_core[t*394+n, kd*128+k]
    xt = nc.declare_dram_parameter("xt", [128, NT, KD * TN], BF16, isOutput=False)
    b1r = nc.declare_dram_parameter("b1r", [128, KH], FP32, isOutput=False)
    b2r = nc.declare_dram_parameter("b2r", [128, KD], FP32, isOutput=False)
    # y out: row (t*6+dc)*128+k, col n == y_core[t*394+n, dc*128+k]
    yt = nc.declare_dram_parameter("yt", [H, TN], FP32, isOutput=True)

    with TileContext(nc) as tc:
        with (
            tc.tile_pool(name="singles", bufs=1) as singles,
            tc.tile_pool(name="gslab", bufs=2) as gslabp,     # [128,4608] bf16
            tc.tile_pool(name="wring", bufs=4) as wringp,     # [128,2,768] fp32
            tc.tile_pool(name="sgnD", bufs=4) as sgnDp,
            tc.tile_pool(name="t1", bufs=NP) as t1p,          # w1 ternary fp8
            tc.tile_pool(name="t2", bufs=NP) as t2p,          # w2 ternary fp8
            tc.tile_pool(name="xb", bufs=NT) as xbp,          # x bf16, resident
            tc.tile_pool(name="hb", bufs=KH * NT) as hbp,     # gelu out, resident
            tc.tile_pool(name="ysb", bufs=4) as ysbp,
            tc.tile_pool(name="psw", bufs=1, space="PSUM") as pswp,
            tc.tile_pool(name="ps1", bufs=4, space="PSUM") as ps1p,
            tc.tile_pool(name="ps2", bufs=3, space="PSUM") as ps2p,
        ):
            # ---- gpsimd library pre-warm (partition_all_reduce is a custom op)
            dmy = singles.tile([128, 1], FP32, tag="dmy")
            nc.gpsimd.memset(dmy, 0.0)
            dmy2 = singles.tile([128, 1], FP32, tag="dmy2")
            nc.gpsimd.partition_all_reduce(dmy2, dmy, channels=128,
                                           reduce_op=bass_isa.ReduceOp.add)

            # ---- biases (gpsimd queue; tiny)
            b1sb = singles.tile([128, KH], FP32, tag="b1sb")
            nc.gpsimd.dma_start(out=b1sb, in_=b1r[:, :])
            b2sb = singles.tile([128, KD], FP32, tag="b2sb")
            nc.gpsimd.dma_start(out=b2sb, in_=b2r[:, :])

            # ---- PE warmup: one long accumulation group, back-to-back MMs
            dmw = singles.tile([128, TN], BF16, tag="dmw")
            nc.vector.memset(dmw, 0.0)
            wps = pswp.tile([128, TN], FP32, tag="wps")
            for i in range(NDUM):
                nc.tensor.matmul(wps, dmw[:, 0:128], dmw,
                                 start=(i == 0), stop=(i == NDUM - 1))

            def gamma_chain(acc, n_cols, tag):
                """|w| col-sums [128,n] -> (g/2, -g/2) broadcast tiles [128,1]"""
                rsum = singles.tile([128, 1], FP32, tag=tag + "_rs")
                nc.vector.tensor_reduce(out=rsum[:, 0:1], in_=acc[:, 0:n_cols],
                                        axis=AxX, op=Alu.add)
                allr = singles.tile([128, 1], FP32, tag=tag + "_ar")
                nc.gpsimd.partition_all_reduce(allr, rsum, channels=128,
                                               reduce_op=bass_isa.ReduceOp.add)
                gf = singles.tile([128, 1], FP32, tag=tag + "_gf")
                nc.vector.tensor_scalar(
                    out=gf, in0=allr, scalar1=1.0 / (D * H),
                    scalar2=EPS, op0=Alu.mult, op1=Alu.add)
                gh = singles.tile([128, 1], FP32, tag=tag + "_gh")
                nc.vector.tensor_scalar_mul(gh, gf, 0.5)
                gn = singles.tile([128, 1], FP32, tag=tag + "_gn")
                nc.vector.tensor_scalar_mul(gn, gf, -0.5)
                return gh, gn

            def gamma_pass(gdram, acc_tag):
                """bf16 pre-read of a weight matrix -> per-partition |w| sums"""
                acc = singles.tile([128, 4], FP32, tag=acc_tag)
                for i in range(4):
                    gt = gslabp.tile([128, KD * D], BF16, tag="gslab")
                    nc.sync.dma_start(out=gt, in_=gdram[:, i, :])
                    nc.vector.tensor_reduce(out=acc[:, i:i + 1], in_=gt,
                                            axis=AxX, op=Alu.add,
                                            apply_absolute_value=True)
                return acc

            def quant_dve(wf, t, gh, gn):
                a = sgnDp.tile(list(wf.shape), FP8, tag="sgnD")
                b = sgnDp.tile(list(wf.shape), FP8, tag="sgnD")
                nc.vector.tensor_scalar(out=a, in0=wf, scalar1=gh[:, 0:1],
                                        scalar2=2.0, op0=Alu.is_ge,
                                        op1=Alu.mult)
                nc.vector.tensor_scalar(out=b, in0=wf, scalar1=gn[:, 0:1],
                                        scalar2=2.0, op0=Alu.is_le,
                                        op1=Alu.mult)
                nc.vector.tensor_sub(out=t, in0=a, in1=b)

            # ---- gamma1 from bf16 pre-pass
            acc1 = gamma_pass(w1g, "acc1")
            g1h, g1n = gamma_chain(acc1, 4, "g1")

            # ---- w1 fp32 pair stream + x loads (sync queue ring order)
            w1f = [None] * NP
            wf0 = wringp.tile([128, 2, D], FP32, tag="wf")
            w1f[0] = wf0
            nc.sync.dma_start(out=wf0, in_=w1p[0:256, :]
                              .rearrange("(b p) f -> p b f", p=128))

            xb = [None] * NT
            for t in (0, 1):
                xbt = xbp.tile([128, KD, TN], BF16, tag="xb")
                xb[t] = xbt
                nc.sync.dma_start(out=xbt, in_=xt[:, t, :]
                                  .rearrange("p (b f) -> p b f", f=TN))

            for pr in range(1, NP):
                wfh = wringp.tile([128, 2, D], FP32, tag="wf")
                w1f[pr] = wfh
                nc.sync.dma_start(out=wfh, in_=w1p[pr * 256:(pr + 1) * 256, :]
                                  .rearrange("(b p) f -> p b f", p=128))

            # ---- quant (DVE) + fc1 interleaved, lookahead 2 pairs
            t1 = [None] * NP

            def quant1(pr):
                t = t1p.tile([128, 2, D], FP8, tag="t1")
                quant_dve(w1f[pr], t, g1h, g1n)
                t1[pr] = t

            hbt = {}

            def fc1(hc, ts):
                for t in ts:
                    ps = ps1p.tile([128, TN], FP32, tag="ps1")
                    for kd in range(KD):
                        nc.tensor.matmul(
                            ps, t1[hc // 2][:, hc % 2, kd * 128:(kd + 1) * 128],
                            xb[t][:, kd, :],
                            start=(kd == 0), stop=(kd == KD - 1))
                    hbv = hbp.tile([128, TN], BF16, tag="hb")
                    nc.scalar.activation(hbv, ps, Act.Gelu,
                                         bias=b1sb[:, hc:hc + 1],
                                         scale=g1h[:, 0:1])
                    hbt[(hc, t)] = hbv

            LOOK = 2
            for pr in range(LOOK):
                quant1(pr)
            for pr in range(NP):
                if pr + LOOK < NP:
                    quant1(pr + LOOK)
                for hc in (2 * pr, 2 * pr + 1):
                    fc1(hc, (0, 1))

            # ---- late x tiles, gamma2 pre-pass, w2 fp32 stream (overlap fc1)
            for t in (2, 3):
                xbt = xbp.tile([128, KD, TN], BF16, tag="xb")
                xb[t] = xbt
                nc.sync.dma_start(out=xbt, in_=xt[:, t, :]
                                  .rearrange("p (b f) -> p b f", f=TN))

            acc2 = gamma_pass(w2g, "acc2")
            g2h, g2n = gamma_chain(acc2, 4, "g2")

            w2f = [None] * NP
            for pr in range(NP):
                wfk = wringp.tile([128, 2, D], FP32, tag="wf")
                w2f[pr] = wfk
                nc.sync.dma_start(out=wfk, in_=w2p[pr * 256:(pr + 1) * 256, :]
                                  .rearrange("(b p) f -> p b f", p=128))

            t2 = [None] * NP

            def quant2(pr):
                t = t2p.tile([128, 2, D], FP8, tag="t2")
                quant_dve(w2f[pr], t, g2h, g2n)
                t2[pr] = t

            for pr in range(LOOK):
                quant2(pr)
            for pr in range(NP):
                if pr + LOOK < NP:
                    quant2(pr + LOOK)
                for hc in (2 * pr, 2 * pr + 1):
                    fc1(hc, (2, 3))

            # ---- fc2: everything resident, PE back-to-back; y out on gpsimd q
            for t in range(NT):
                for dc in range(KD):
                    ps = ps2p.tile([128, TN], FP32, tag="ps2")
                    for kh in range(KH):
                        nc.tensor.matmul(
                            ps, t2[kh // 2][:, kh % 2, dc * 128:(dc + 1) * 128],
                            hbt[(kh, t)],
                            start=(kh == 0), stop=(kh == KH - 1))
                    ysb = ysbp.tile([128, TN], FP32, tag="ysb")
                    nc.vector.tensor_scalar(
                        out=ysb, in0=ps, scalar1=g2h[:, 0:1],
                        scalar2=b2sb[:, dc:dc + 1],
                        op0=Alu.mult, op1=Alu.add)
                    nc.gpsimd.dma_start(
                        out=yt[(t * KD + dc) * 128:(t * KD + dc + 1) * 128, :],
                        in_=ysb)

    nc.compile()
    return nc


_NC = None


def _get_nc():
    global _NC
    if _NC is None:
        _NC = build()
    return _NC


def kernel(x, w1, b1, w2, b2, _trace=False, _trace_kwargs=None):
    nc = _get_nc()
    x = np.asarray(x, dtype=np.float32)
    w1 = np.asarray(w1, dtype=np.float32)
    b1 = np.asarray(b1, dtype=np.float32)
    w2 = np.asarray(w2, dtype=np.float32)
    b2 = np.asarray(b2, dtype=np.float32)

    # w1 -> hc-blocked layout [3072, 768]: row hc*128+k, col kd*128+m
    w1p = np.ascontiguousarray(
        w1.reshape(KH, 128, KD, 128).transpose(0, 3, 2, 1).reshape(H, D))
    w2p = np.ascontiguousarray(w2.T)                    # [3072, 768]
    # partition-major bf16 slabs [128, 4, 4608] for the gamma passes
    w1g = np.ascontiguousarray(
        w1p.reshape(4, KD, 128, D).transpose(2, 0, 1, 3).reshape(128, 4, KD * D)
    ).astype(ml_dtypes.bfloat16)
    w2g = np.ascontiguousarray(
        w2p.reshape(4, KD, 128, D).transpose(2, 0, 1, 3).reshape(128, 4, KD * D)
    ).astype(ml_dtypes.bfloat16)
    b1r = np.ascontiguousarray(b1.reshape(KH, 128).T)   # [128, 24]
    b2r = np.ascontiguousarray(b2.reshape(KD, 128).T)   # [128, 6]

    x2 = x.reshape(TOK, D)
    in_maps = []
    for c in range(N_CORES):
        xc = x2[c * TOK_PER:(c + 1) * TOK_PER]          # [1576, 768]
        # [128, t, kd*394]: xtc[k, t, kd*TN+n] = xc[t*394+n, kd*128+k]
        xtc = np.ascontiguousarray(
            xc.reshape(NT, TN, KD, 128).transpose(3, 0, 2, 1).reshape(128, NT, KD * TN)
        ).astype(ml_dtypes.bfloat16)
        in_maps.append({
            "xt": xtc, "w1g": w1g, "w1p": w1p, "w2g": w2g, "w2p": w2p,
            "b1r": b1r, "b2r": b2r,
        })
    out = run_bass_kernel_spmd(nc, in_maps, list(range(N_CORES)),
                               trace=_trace, **(_trace_kwargs or {}))
    res = out.results
    y = np.empty((TOK, D), dtype=np.float32)
    for c in range(N_CORES):
        ytc = res[c]["yt"]                              # [3072, 394]
        y[c * TOK_PER:(c + 1) * TOK_PER] = (
            ytc.reshape(NT, KD, 128, TN).transpose(0, 3, 1, 2).reshape(TOK_PER, D))
    y = y.reshape(B, S, D)
    if _trace:
        return y, out
    return y
